# revision 1
# baseline (speedup 1.0000x reference)
"""ContraNorm Trainium2 kernel: out = 1.2*x - 0.2 * softmax(xn @ xn^T) @ x per batch.

Full input x [8, 2048, 512] f32; batch dim sharded across 8 NeuronCores
(data-parallel, no collectives). Each core runs an identical Bass/Tile program
on its [2048, 512] slice.

Per-core pipeline (bf16 matmul inputs, fp32 PSUM accumulation):
  setup: load x; row sum-of-squares via ACT Square w/ accum_out; rn = 1/sqrt;
         xn = x * rn cast to bf16; PE-transpose xn -> xnT [128, 4, 2048]
         (d on partitions); xe bf16 [128, 16, 520] = x chunks + ones col at 512.
  per 256-row block:
    MM1: S^T[n, m] chunks in PSUM (n on partitions, m on free) -- this makes the
         softmax numerator tiles directly usable as MM2's stationary operand,
         avoiding any attn transpose.
    exp: ACT (no max subtraction needed: sim values are cosines in [-1, 1]).
    MM2: per 128-row half: O = expST.T @ [x | 1] -> PSUM; the ones column
         yields the softmax denominator D at PSUM col 768.
    final: out = 1.2*x + (-0.2/D) * O on DVE; DMA out.
"""

import sys

if "/opt/trn_rl_repo" not in sys.path:
    sys.path.insert(0, "/opt/trn_rl_repo")

from contextlib import ExitStack

import numpy as np

import concourse.bass as bass
import concourse.tile as tile
import concourse.mybir as mybir
from concourse import bacc
from concourse.masks import make_identity
from concourse.bass_utils import run_bass_kernel_spmd

F32 = mybir.dt.float32
BF16 = mybir.dt.bfloat16
AF = mybir.ActivationFunctionType
ALU = mybir.AluOpType

B = 8
P = 128
N = 2048
D = 512
NT = N // P      # 16 row tiles
DS = D // P      # 4 d subtiles
MB = 256         # m superblock (2 row tiles per iter)
ITERS = N // MB  # 8
WCH = 4          # n-chunks per psum wave (2 PSUM banks per wave tile)
WAVES = NT // WCH  # 8

VARIANT = ""  # debug bisect switches, comma-separated
FP8 = mybir.dt.float8e4
USE_FP8 = True  # fp8e4m3 matmul inputs + DoubleRow perf mode (2x PE throughput)


def contranorm_body(ctx: ExitStack, tc: tile.TileContext, out_ap: bass.AP, x_ap: bass.AP):
    nc = tc.nc

    singles = ctx.enter_context(tc.tile_pool(name="singles", bufs=1))
    scratch = ctx.enter_context(tc.tile_pool(name="scratch", bufs=3))
    stats = ctx.enter_context(tc.tile_pool(name="stats", bufs=8))
    xnpool = ctx.enter_context(tc.tile_pool(name="xnpool", bufs=4))

    # persistent tensors
    MDT = FP8 if USE_FP8 else BF16  # matmul input dtype
    xf = singles.tile([P, NT, D], F32)        # x, natural layout (n on partitions)
    xe = singles.tile([P, NT, D + 16], MDT)   # x + ones column at [.., D]
    xnT = singles.tile([P, DS, N], MDT)       # xn transposed (d on partitions)
    # PE transpose path stays bf16 (fp8 transpose needs stride-2 psum writes);
    # the PSUM->SBUF copy casts to MDT.
    ident = singles.tile([P, P], BF16)
    make_identity(nc, ident)
    nc.vector.memset(xe[:, :, D:D + 1], 1.0)

    # PSUM budget (8 banks total, all pools coexist so the scheduler can
    # overlap setup transposes with early main-loop matmuls):
    #   tpsum 2 x 1 bank, psumS 2 x 2 banks, psumO 1 x 2 banks.
    tpsum = ctx.enter_context(tc.tile_pool(name="tpsum", bufs=2, space="PSUM"))
    psumS = ctx.enter_context(tc.tile_pool(name="psumS", bufs=2, space="PSUM"))
    psumO = ctx.enter_context(tc.tile_pool(name="psumO", bufs=1, space="PSUM"))

    # ---------------- setup: norms, xn, transpose ----------------
    variants = set(VARIANT.split(","))
    for i in range(NT):
        nc.sync.dma_start(xf[:, i, :], x_ap[i * P:(i + 1) * P, :])
        # mean/var via DVE bn_stats; ssq/D = var + mean^2
        bst = scratch.tile([P, nc.vector.BN_STATS_DIM], F32, tag="bst")
        nc.vector.bn_stats(bst, xf[:, i, :])
        mv = stats.tile([P, nc.vector.BN_AGGR_DIM], F32, tag="mv")
        nc.vector.bn_aggr(mv, bst)
        vpm = stats.tile([P, 1], F32, tag="vpm")
        nc.vector.tensor_tensor(vpm, mv[:, 0:1], mv[:, 0:1], op=ALU.mult)
        nc.vector.tensor_add(vpm, vpm, mv[:, 1:2])
        # rn = 1/sqrt(vpm * D)  (norms ~22.6 >> eps, the eps clamp is a no-op)
        nrm = stats.tile([P, 1], F32, tag="nrm")
        nc.scalar.activation(nrm, vpm, AF.Sqrt, scale=float(D))
        rn = stats.tile([P, 1], F32, tag="rn")
        nc.vector.reciprocal(rn, nrm)
        # xe chunk: cast x to bf16
        if "actcast" in variants:
            nc.scalar.copy(xe[:, i, 0:D], xf[:, i, :])
        else:
            nc.gpsimd.tensor_copy(xe[:, i, 0:D], xf[:, i, :])
        # xn = x * rn -> bf16 (DVE)
        xn = xnpool.tile([P, D], BF16, tag="xn")
        nc.vector.tensor_scalar_mul(xn, xf[:, i, :], rn)
        # transpose xn into xnT columns [i*P, (i+1)*P); all 4 d-chunks land in
        # one psum bank, one batched copy (engine picked by the scheduler)
        pt = tpsum.tile([P, DS, P], BF16, tag="pt")
        for dc in range(DS):
            nc.tensor.transpose(pt[:, dc, :], xn[:, dc * P:(dc + 1) * P], ident)
        nc.any.tensor_copy(xnT[:, :, i * P:(i + 1) * P], pt)

    # ---------------- main loop ----------------
    epool = ctx.enter_context(tc.tile_pool(name="epool", bufs=3))
    opool = ctx.enter_context(tc.tile_pool(name="opool", bufs=3))

    for it in range(ITERS):
        mlo = it * MB
        expST = epool.tile([P, NT, MB], MDT, tag="expST")
        for w in range(WAVES):
            ps = psumS.tile([P, WCH, MB], F32, tag="ps")  # 1 bank
            for c in range(WCH):
                j = w * WCH + c  # n-chunk index
                if USE_FP8:
                    for g in range(DS // 2):
                        nc.tensor.matmul(
                            ps[:, c, :],
                            lhsT=xnT[:, 2 * g:2 * g + 2, j * P:(j + 1) * P],
                            rhs=xnT[:, 2 * g:2 * g + 2, mlo:mlo + MB],
                            start=(g == 0),
                            stop=(g == DS // 2 - 1),
                            perf_mode=mybir.MatmulPerfMode.DoubleRow,
                        )
                else:
                    for ds in range(DS):
                        nc.tensor.matmul(
                            ps[:, c, :],
                            lhsT=xnT[:, ds, j * P:(j + 1) * P],
                            rhs=xnT[:, ds, mlo:mlo + MB],
                            start=(ds == 0),
                            stop=(ds == DS - 1),
                        )
            # exp of the whole wave in one ACT instruction
            nc.scalar.activation(expST[:, w * WCH:(w + 1) * WCH, :], ps, AF.Exp)

        for h in range(2):
            i = it * 2 + h  # output row-tile index
            po = psumO.tile([P, 1024], F32, tag="po")  # 2 banks
            if USE_FP8:
                for g in range(NT // 2):
                    lhsT = expST[:, 2 * g:2 * g + 2, h * P:(h + 1) * P]
                    nc.tensor.matmul(po[:, 0:256], lhsT, xe[:, 2 * g:2 * g + 2, 0:256],
                                     start=(g == 0), stop=(g == NT // 2 - 1),
                                     perf_mode=mybir.MatmulPerfMode.DoubleRow)
                    nc.tensor.matmul(po[:, 512:512 + 257], lhsT,
                                     xe[:, 2 * g:2 * g + 2, 256:D + 1],
                                     start=(g == 0), stop=(g == NT // 2 - 1),
                                     perf_mode=mybir.MatmulPerfMode.DoubleRow)
            else:
                for k in range(NT):
                    lhsT = expST[:, k, h * P:(h + 1) * P]
                    nc.tensor.matmul(po[:, 0:256], lhsT, xe[:, k, 0:256],
                                     start=(k == 0), stop=(k == NT - 1))
                    nc.tensor.matmul(po[:, 512:512 + 257], lhsT, xe[:, k, 256:D + 1],
                                     start=(k == 0), stop=(k == NT - 1))
            # s = -0.2 / D  (D at psum col 768)
            sD = stats.tile([P, 1], F32, tag="sD")
            nc.vector.tensor_scalar_mul(sD, po[:, 768:769], -5.0)
            rD = stats.tile([P, 1], F32, tag="rD")
            nc.vector.reciprocal(rD, sD)
            # tmp = O * s ; O cols are [0:256] and [512:768]
            tmp = opool.tile([P, 2, 256], F32, tag="tmp")
            po3 = po.rearrange("p (b c) -> p b c", b=2, c=512)[:, :, 0:256]
            nc.vector.tensor_scalar_mul(tmp, po3, rD)
            # out = x * 1.2 + tmp
            ob = opool.tile([P, D], F32, tag="ob")
            nc.vector.scalar_tensor_tensor(
                ob, xf[:, i, :], 1.2, tmp.rearrange("p b c -> p (b c)"),
                op0=ALU.mult, op1=ALU.add)
            nc.sync.dma_start(out_ap[i * P:(i + 1) * P, :], ob)


def build_nc(repeats: int = 1, loop: int = 0):
    """Build + compile the per-core Bass program. `repeats` re-emits the body
    (sharing pools/SBUF); `loop` wraps the body in a For_i hardware loop --
    both are for steady-state timing measurements."""
    nc = bacc.Bacc("TRN2", target_bir_lowering=False, debug=False, enable_asserts=False)
    x = nc.dram_tensor("x", [N, D], F32, kind="ExternalInput").ap()
    out = nc.dram_tensor("out", [N, D], F32, kind="ExternalOutput").ap()
    with tile.TileContext(nc) as tc:
        if loop:
            with ExitStack() as ctx:
                with tc.For_i(0, loop, 1):
                    contranorm_body(ctx, tc, out, x)
        else:
            for _ in range(repeats):
                with ExitStack() as ctx:
                    contranorm_body(ctx, tc, out, x)
    nc.compile()
    return nc


_nc_cache = {}


def kernel(x: np.ndarray) -> np.ndarray:
    assert x.shape == (B, N, D), x.shape
    x = np.ascontiguousarray(x, dtype=np.float32)
    if "nc" not in _nc_cache:
        _nc_cache["nc"] = build_nc()
    nc = _nc_cache["nc"]
    in_maps = [{"x": x[i]} for i in range(B)]
    res = run_bass_kernel_spmd(nc, in_maps, core_ids=list(range(B)))
    return np.stack([r["out"] for r in res.results], axis=0)



# revision 3
# speedup vs baseline: 1.1093x; 1.1093x over previous
"""ContraNorm Trainium2 kernel: out = 1.2*x - 0.2 * softmax(xn @ xn^T) @ x per batch.

Full input x [8, 2048, 512] f32; batch dim sharded across 8 NeuronCores
(data-parallel, no collectives). Each core runs an identical Bass/Tile program
on its [2048, 512] slice.

v2: exploits symmetry of sim = xn @ xn^T. Row-chunk orientation: chunk c
(rows 128c..128c+127 on partitions) computes sim columns b >= 128c only
(upper trapezoid, 144/256 blocks). The lower-left blocks are mirrors:
E2[:, j, cP:(c+1)P] = T(E2[:, c, jP:(j+1)P]) for j > c, produced by PE
matmul-transpose (lhsT = E-block, rhs = fp8 identity -> f32 PSUM) plus a
batched cast-copy back to fp8 SBUF. This halves both MM1 PE work and ACT
exp work (the two largest engine costs in the cost-model timeline).

Setup: sum-of-squares via ACT Square+accum_out (keeps DVE free; Square/Exp/
Copy share one ACT table set so there is no table thrash); one batched Sqrt;
xn = x * rn in bf16 on DVE; xnT via PE transpose (cast to fp8 on copy-out).

MM2 (out rows m in tile h): lhsT = E2[:, 2g:2g+2, hP:(h+1)P] (n on
partitions), rhs = xe = [x | ones] fp8, DoubleRow; po[:, 0:512] = attn-num,
po[:, 512] = softmax denom D. Final: tmp = po * (-0.2/D) on ACT (Copy with
per-partition scale), out = 1.2*x + tmp on DVE.
"""

import sys

if "/opt/trn_rl_repo" not in sys.path:
    sys.path.insert(0, "/opt/trn_rl_repo")

from contextlib import ExitStack

import numpy as np

import concourse.bass as bass
import concourse.tile as tile
import concourse.mybir as mybir
from concourse import bacc
from concourse.masks import make_identity
from concourse.bass_utils import run_bass_kernel_spmd

F32 = mybir.dt.float32
BF16 = mybir.dt.bfloat16
FP8 = mybir.dt.float8e4
AF = mybir.ActivationFunctionType
ALU = mybir.AluOpType
DR = mybir.MatmulPerfMode.DoubleRow

B = 8
P = 128
N = 2048
D = 512
NT = N // P      # 16 row chunks
DS = D // P      # 4 d subtiles

VARIANT = ""  # debug bisect switches, comma-separated


def pieces_for_chunk(c):
    """b-ranges (512-bank-bounded) covering [128c, 2048)."""
    out = []
    b = 128 * c
    while b < N:
        e = min((b // 512 + 1) * 512, N)
        out.append((b, e))
        b = e
    return out


def contranorm_body(ctx: ExitStack, tc: tile.TileContext, out_ap: bass.AP, x_ap: bass.AP):
    nc = tc.nc
    variants = set(VARIANT.split(","))

    singles = ctx.enter_context(tc.tile_pool(name="singles", bufs=1))
    scratch = ctx.enter_context(tc.tile_pool(name="scratch", bufs=3))
    stats = ctx.enter_context(tc.tile_pool(name="stats", bufs=8))
    xnpool = ctx.enter_context(tc.tile_pool(name="xnpool", bufs=4))

    # persistent tensors
    xf = singles.tile([P, NT, D], F32)        # x, natural layout (n on partitions)
    xe = singles.tile([P, NT, D + 16], FP8)   # x + ones column at [.., D]
    xnT = singles.tile([P, DS, N], FP8)       # xn transposed (d on partitions)
    E2 = singles.tile([P, NT, N], FP8)        # exp(sim); E2[p, c, b] = exp(sim)[128c+p, b]
    ssqA = singles.tile([P, NT], F32)         # per-row sum of squares (col i = tile i)
    nrmA = singles.tile([P, NT], F32)
    rnA = singles.tile([P, NT], F32)
    identB = singles.tile([P, P], BF16)
    identE = singles.tile([P, P], FP8)
    make_identity(nc, identB)
    make_identity(nc, identE)
    nc.vector.memset(xe[:, :, D:D + 1], 1.0)

    # PSUM budget (8 banks): psumS 2x1, psumM 2x1 (shared w/ setup transposes),
    # psumO 2x2.
    psumS = ctx.enter_context(tc.tile_pool(name="psumS", bufs=2, space="PSUM"))
    psumM = ctx.enter_context(tc.tile_pool(name="psumM", bufs=2, space="PSUM"))
    psumO = ctx.enter_context(tc.tile_pool(name="psumO", bufs=2, space="PSUM"))

    # ---------------- setup: norms, xn, transpose ----------------
    for i in range(NT):
        nc.sync.dma_start(xf[:, i, :], x_ap[i * P:(i + 1) * P, :])
        # ssq via ACT Square + accum (exp_and_friends table: no thrash w/ Exp)
        sq = scratch.tile([P, D], F32, tag="sq")
        nc.scalar.activation(sq, xf[:, i, :], AF.Square,
                             accum_out=ssqA[:, i:i + 1])
        # xe chunk: cast x to fp8
        nc.gpsimd.tensor_copy(xe[:, i, 0:D], xf[:, i, :])
    # rn = 1/sqrt(ssq), batched
    nc.scalar.activation(nrmA, ssqA, AF.Sqrt)
    nc.vector.reciprocal(rnA, nrmA)
    for i in range(NT):
        xn = xnpool.tile([P, D], BF16, tag="xn")
        nc.vector.tensor_scalar_mul(xn, xf[:, i, :], rnA[:, i:i + 1])
        pt = psumM.tile([P, DS, P], BF16, tag="pm")
        for dc in range(DS):
            nc.tensor.transpose(pt[:, dc, :], xn[:, dc * P:(dc + 1) * P], identB)
        nc.any.tensor_copy(xnT[:, :, i * P:(i + 1) * P], pt)

    # ---------------- main loop: one row-chunk c at a time ----------------
    tmppool = ctx.enter_context(tc.tile_pool(name="tmppool", bufs=3))
    opool = ctx.enter_context(tc.tile_pool(name="opool", bufs=3))

    for c in range(NT):
        # MM1 + exp over the trapezoid pieces b in [128c, 2048)
        for (b0, b1) in pieces_for_chunk(c):
            w = b1 - b0
            ps = psumS.tile([P, w], F32, tag="ps")
            for g in range(2):
                nc.tensor.matmul(
                    ps,
                    lhsT=xnT[:, 2 * g:2 * g + 2, c * P:(c + 1) * P],
                    rhs=xnT[:, 2 * g:2 * g + 2, b0:b1],
                    start=(g == 0), stop=(g == 1), perf_mode=DR)
            nc.scalar.activation(E2[:, c, b0:b1], ps, AF.Exp)
            # mirrors for the full blocks inside this piece: j > c
            j0 = max(c + 1, (b0 + P - 1) // P)
            j1 = b1 // P
            if j1 > j0:
                nb = j1 - j0
                pm = psumM.tile([P, nb, P], F32, tag="pm")
                for t in range(nb):
                    j = j0 + t
                    nc.tensor.matmul(
                        pm[:, t, :],
                        lhsT=E2[:, c, j * P:(j + 1) * P],
                        rhs=identE, start=True, stop=True)
                nc.any.tensor_copy(E2[:, j0:j1, c * P:(c + 1) * P], pm)

        # MM2 for out row-tile h = c (all needed E2 slices now exist)
        h = c
        po = psumO.tile([P, 1024], F32, tag="po")  # 2 banks; [0:512]=O, [512]=D
        for g in range(NT // 2):
            lhsT = E2[:, 2 * g:2 * g + 2, h * P:(h + 1) * P]
            nc.tensor.matmul(po[:, 0:D], lhsT, xe[:, 2 * g:2 * g + 2, 0:D],
                             start=(g == 0), stop=(g == NT // 2 - 1), perf_mode=DR)
            nc.tensor.matmul(po[:, D:D + 1], lhsT, xe[:, 2 * g:2 * g + 2, D:D + 1],
                             start=(g == 0), stop=(g == NT // 2 - 1), perf_mode=DR)
        # s = -0.2 / D
        sD = stats.tile([P, 1], F32, tag="sD")
        nc.vector.tensor_scalar_mul(sD, po[:, D:D + 1], -5.0)
        rD = stats.tile([P, 1], F32, tag="rD")
        nc.vector.reciprocal(rD, sD)
        # tmp = O * s (ACT Copy w/ per-partition scale, PSUM -> SBUF)
        tmp = tmppool.tile([P, D], F32, tag="tmp")
        if "tmpdve" in variants:
            nc.vector.tensor_scalar_mul(tmp, po[:, 0:D], rD)
        else:
            nc.scalar.activation(tmp, po[:, 0:D], AF.Copy, scale=rD)
        # out = x * 1.2 + tmp
        ob = opool.tile([P, D], F32, tag="ob")
        nc.vector.scalar_tensor_tensor(
            ob, xf[:, h, :], 1.2, tmp, op0=ALU.mult, op1=ALU.add)
        nc.sync.dma_start(out_ap[h * P:(h + 1) * P, :], ob)


def build_nc(repeats: int = 1, loop: int = 0):
    """Build + compile the per-core Bass program. `repeats` re-emits the body
    (sharing pools/SBUF); `loop` wraps the body in a For_i hardware loop --
    both are for steady-state timing measurements."""
    nc = bacc.Bacc("TRN2", target_bir_lowering=False, debug=False, enable_asserts=False)
    x = nc.dram_tensor("x", [N, D], F32, kind="ExternalInput").ap()
    out = nc.dram_tensor("out", [N, D], F32, kind="ExternalOutput").ap()
    with tile.TileContext(nc) as tc:
        if loop:
            with ExitStack() as ctx:
                with tc.For_i(0, loop, 1):
                    contranorm_body(ctx, tc, out, x)
        else:
            for _ in range(repeats):
                with ExitStack() as ctx:
                    contranorm_body(ctx, tc, out, x)
    nc.compile()
    return nc


_nc_cache = {}


def kernel(x: np.ndarray) -> np.ndarray:
    assert x.shape == (B, N, D), x.shape
    x = np.ascontiguousarray(x, dtype=np.float32)
    if "nc" not in _nc_cache:
        _nc_cache["nc"] = build_nc()
    nc = _nc_cache["nc"]
    in_maps = [{"x": x[i]} for i in range(B)]
    res = run_bass_kernel_spmd(nc, in_maps, core_ids=list(range(B)))
    return np.stack([r["out"] for r in res.results], axis=0)


# revision 15
# speedup vs baseline: 1.3018x; 1.1735x over previous
"""ContraNorm Trainium2 kernel: out = 1.2*x - 0.2 * softmax(xn @ xn^T) @ x per batch.

Full input x [8, 2048, 512] f32; batch dim sharded across 8 NeuronCores
(data-parallel, no collectives). Each core runs an identical Bass/Tile program
on its [2048, 512] slice.

v4: exploits symmetry of sim = xn @ xn^T. Row-chunk orientation: chunk c
(rows 128c..128c+127 on partitions) computes sim columns b >= 128c only
(upper trapezoid, 144/256 blocks). The lower-left blocks are mirrors:
E2[:, j, cP:(c+1)P] = T(E2[:, c, jP:(j+1)P]) for j > c, produced by PE
matmul-transpose (lhsT = E-block, rhs = fp8 identity -> f32 PSUM) plus a
batched cast-copy back to fp8 SBUF. This halves both MM1 PE work and ACT
exp work (the two largest engine costs in the cost-model timeline).

Work is spread across engines to keep the setup and main-loop phases
balanced (cost-model gantt driven):
  setup: input DMAs alternate SP/ACT/DVE queues; ssq split ACT (Square +
  accum) / DVE (stt + accum); sqrt in two batches of 8 (halves the barrier);
  xn alternates DVE/ACT (Copy w/ per-partition scale); xnT copy-out round-
  robins DVE/ACT/Pool; xe cast (only needed by MM2) runs on Pool.
  main: exp on ACT; mirror copies alternate DVE/Pool; attn scale (po *
  -0.2/D) alternates ACT/DVE; combine on DVE.

Pools are created once and shared by all repeat bodies so repeated bodies
pipeline without per-body drains (steady-state measurement matches the
single-body program the grader runs).
"""

import sys

if "/opt/trn_rl_repo" not in sys.path:
    sys.path.insert(0, "/opt/trn_rl_repo")

from contextlib import ExitStack

import numpy as np

import concourse.bass as bass
import concourse.tile as tile
import concourse.mybir as mybir
from concourse import bacc
from concourse.masks import make_identity
from concourse.bass_utils import run_bass_kernel_spmd

F32 = mybir.dt.float32
BF16 = mybir.dt.bfloat16
FP8 = mybir.dt.float8e4
AF = mybir.ActivationFunctionType
ALU = mybir.AluOpType
DR = mybir.MatmulPerfMode.DoubleRow

B = 8
P = 128
N = 2048
D = 512
NT = N // P      # 16 row chunks
DS = D // P      # 4 d subtiles

VARIANT = ""  # debug bisect switches, comma-separated


def spans_for_chunk(c):
    """(b0, b1) spans covering [128c, 2048), each within one 512-f32 PSUM
    bank."""
    out = []
    b = 128 * c
    while b < N:
        e = min((b // 512 + 1) * 512, N)
        out.append((b, e))
        b = e
    return out


def make_pools(ctx: ExitStack, tc: tile.TileContext):
    pools = {}
    pools["singles"] = ctx.enter_context(tc.tile_pool(name="singles", bufs=1))
    pools["scratch"] = ctx.enter_context(tc.tile_pool(name="scratch", bufs=3))
    pools["stats"] = ctx.enter_context(tc.tile_pool(name="stats", bufs=8))
    pools["xnpool"] = ctx.enter_context(tc.tile_pool(name="xnpool", bufs=4))
    pools["tmppool"] = ctx.enter_context(tc.tile_pool(name="tmppool", bufs=3))
    pools["opool"] = ctx.enter_context(tc.tile_pool(name="opool", bufs=3))
    # PSUM budget (8 banks): psumS 4x1 (512-wide MM1+exp tiles; deep ring so
    # MM1 can run well ahead of the slower exp drain), psumM 2x1 (setup
    # transposes + mirror transposes, shared tag), psumO 1x2 (MM2 numerator +
    # denominator at col 512).
    pools["psumS"] = ctx.enter_context(tc.tile_pool(name="psumS", bufs=4, space="PSUM"))
    pools["psumM"] = ctx.enter_context(tc.tile_pool(name="psumM", bufs=2, space="PSUM"))
    pools["psumO"] = ctx.enter_context(tc.tile_pool(name="psumO", bufs=1, space="PSUM"))
    return pools


def contranorm_body(pools, tc: tile.TileContext, out_ap: bass.AP, x_ap: bass.AP,
                    first: bool):
    nc = tc.nc
    variants = set(VARIANT.split(","))

    singles = pools["singles"]
    scratch = pools["scratch"]
    stats = pools["stats"]
    xnpool = pools["xnpool"]
    tmppool = pools["tmppool"]
    opool = pools["opool"]
    psumS, psumM = pools["psumS"], pools["psumM"]
    psumO = pools["psumO"]

    # persistent tensors (same tiles every repeat; dependency-tracked)
    if first:
        xf = singles.tile([P, NT, D], F32, tag="xf")
        xe = singles.tile([P, NT, D + 16], FP8, tag="xe")
        xnT = singles.tile([P, DS, N], FP8, tag="xnT")
        E2 = singles.tile([P, NT, N], FP8, tag="E2")
        ssqA = singles.tile([P, NT], F32, tag="ssqA")
        nrmA = singles.tile([P, NT], F32, tag="nrmA")
        rnA = singles.tile([P, NT], F32, tag="rnA")
        identB = singles.tile([P, P], BF16, tag="identB")
        identE = singles.tile([P, P], FP8, tag="identE")
        pools.update(xf=xf, xe=xe, xnT=xnT, E2=E2, ssqA=ssqA, nrmA=nrmA,
                     rnA=rnA, identB=identB, identE=identE)
        make_identity(nc, identB)
        make_identity(nc, identE)
        nc.vector.memset(xe[:, :, D:D + 1], 1.0)
    xf, xe, xnT, E2 = pools["xf"], pools["xe"], pools["xnT"], pools["E2"]
    ssqA, nrmA, rnA = pools["ssqA"], pools["nrmA"], pools["rnA"]
    identB, identE = pools["identB"], pools["identE"]

    # ---------------- setup: norms, xn, transpose ----------------
    for i in range(NT):
        nc.sync.dma_start(xf[:, i, :], x_ap[i * P:(i + 1) * P, :])
        if i % 2 == 0:
            # ssq via ACT Square + accum_out
            sq = scratch.tile([P, D], F32, tag="sq")
            nc.scalar.activation(sq, xf[:, i, :], AF.Square,
                                 accum_out=ssqA[:, i:i + 1])
        else:
            # ssq via DVE: sq = (x * 1) * x, accum_out = sum(sq)
            sq = scratch.tile([P, D], F32, tag="sq")
            nc.vector.scalar_tensor_tensor(
                sq, xf[:, i, :], 1.0, xf[:, i, :], op0=ALU.mult, op1=ALU.mult,
                accum_out=ssqA[:, i:i + 1])
        # xe chunk (only needed once MM2 starts): Pool
        nc.gpsimd.tensor_copy(xe[:, i, 0:D], xf[:, i, :])
        if i == 7:
            nc.scalar.activation(nrmA[:, 0:8], ssqA[:, 0:8], AF.Sqrt)
            nc.vector.reciprocal(rnA[:, 0:8], nrmA[:, 0:8])
        elif i == 15:
            nc.scalar.activation(nrmA[:, 8:16], ssqA[:, 8:16], AF.Sqrt)
            nc.vector.reciprocal(rnA[:, 8:16], nrmA[:, 8:16])
    copy_engs = [nc.vector.tensor_copy, nc.scalar.copy]
    for i in range(NT):
        xn = xnpool.tile([P, D], BF16, tag="xn")
        if i % 2 == 0:
            nc.vector.tensor_scalar_mul(xn, xf[:, i, :], rnA[:, i:i + 1])
        else:
            nc.scalar.activation(xn, xf[:, i, :], AF.Copy, scale=rnA[:, i:i + 1])
        pt = psumM.tile([P, DS, P], BF16, tag="pm")
        for dc in range(DS):
            nc.tensor.transpose(pt[:, dc, :], xn[:, dc * P:(dc + 1) * P], identB)
        copy_engs[i % 2](xnT[:, :, i * P:(i + 1) * P], pt)

    # ---------------- main loop: one row-chunk c at a time ----------------
    mir_flip = 0
    for c in range(NT):
        # MM1 + exp over the trapezoid spans b in [128c, 2048)
        for (b0, b1) in spans_for_chunk(c):
            w = b1 - b0
            ps = psumS.tile([P, w], F32, tag="ps")
            for g in range(2):
                nc.tensor.matmul(
                    ps,
                    lhsT=xnT[:, 2 * g:2 * g + 2, c * P:(c + 1) * P],
                    rhs=xnT[:, 2 * g:2 * g + 2, b0:b1],
                    start=(g == 0), stop=(g == 1), perf_mode=DR)
            nc.scalar.activation(E2[:, c, b0:b1], ps, AF.Exp)
            # mirrors for the full blocks inside this span: j > c
            j0 = max(c + 1, (b0 + P - 1) // P)
            j1 = b1 // P
            for w0 in range(j0, j1, 4):
                w1 = min(w0 + 4, j1)
                nb = w1 - w0
                pm = psumM.tile([P, nb, P], F32, tag="pm")
                for t in range(nb):
                    j = w0 + t
                    nc.tensor.matmul(
                        pm[:, t, :],
                        lhsT=E2[:, c, j * P:(j + 1) * P],
                        rhs=identE, start=True, stop=True)
                if mir_flip % 2 == 0:
                    nc.vector.tensor_copy(E2[:, w0:w1, c * P:(c + 1) * P], pm)
                else:
                    nc.scalar.copy(E2[:, w0:w1, c * P:(c + 1) * P], pm)
                mir_flip += 1

        # MM2 for out row-tile h = c (all needed E2 slices now exist)
        h = c
        po = psumO.tile([P, 1024], F32, tag="po")  # [0:512]=num, [512]=denom
        for g in range(NT // 2):
            lhsT = E2[:, 2 * g:2 * g + 2, h * P:(h + 1) * P]
            nc.tensor.matmul(po[:, 0:D], lhsT, xe[:, 2 * g:2 * g + 2, 0:D],
                             start=(g == 0), stop=(g == NT // 2 - 1), perf_mode=DR)
            nc.tensor.matmul(po[:, D:D + 1], lhsT, xe[:, 2 * g:2 * g + 2, D:D + 1],
                             start=(g == 0), stop=(g == NT // 2 - 1), perf_mode=DR)
        # s = -0.2 / D
        sD = stats.tile([P, 1], F32, tag="sD")
        nc.vector.tensor_scalar_mul(sD, po[:, D:D + 1], -5.0)
        rD = stats.tile([P, 1], F32, tag="rD")
        nc.vector.reciprocal(rD, sD)
        # tmp = O * s (per-partition scale ptr, PSUM -> SBUF), ACT/DVE split
        tmp = tmppool.tile([P, D], F32, tag="tmp")
        if c % 2 == 0:
            nc.scalar.activation(tmp, po[:, 0:D], AF.Copy, scale=rD)
        else:
            nc.vector.tensor_scalar_mul(tmp, po[:, 0:D], rD)
        # out = x * 1.2 + tmp
        ob = opool.tile([P, D], F32, tag="ob")
        nc.vector.scalar_tensor_tensor(
            ob, xf[:, h, :], 1.2, tmp, op0=ALU.mult, op1=ALU.add)
        nc.scalar.dma_start(out_ap[h * P:(h + 1) * P, :], ob)


def build_nc(repeats: int = 1, loop: int = 0):
    """Build + compile the per-core Bass program. `repeats` re-emits the body
    (sharing pools/SBUF); `loop` wraps the body in a For_i hardware loop --
    both are for steady-state timing measurements."""
    nc = bacc.Bacc("TRN2", target_bir_lowering=False, debug=False, enable_asserts=False)
    x = nc.dram_tensor("x", [N, D], F32, kind="ExternalInput").ap()
    out = nc.dram_tensor("out", [N, D], F32, kind="ExternalOutput").ap()
    with tile.TileContext(nc) as tc:
        with ExitStack() as ctx:
            pools = make_pools(ctx, tc)
            if loop:
                with tc.For_i(0, loop, 1):
                    contranorm_body(pools, tc, out, x, first=True)
            else:
                for r in range(repeats):
                    contranorm_body(pools, tc, out, x, first=(r == 0))
    nc.compile()
    return nc


_nc_cache = {}


def kernel(x: np.ndarray) -> np.ndarray:
    assert x.shape == (B, N, D), x.shape
    x = np.ascontiguousarray(x, dtype=np.float32)
    if "nc" not in _nc_cache:
        _nc_cache["nc"] = build_nc()
    nc = _nc_cache["nc"]
    in_maps = [{"x": x[i]} for i in range(B)]
    res = run_bass_kernel_spmd(nc, in_maps, core_ids=list(range(B)))
    return np.stack([r["out"] for r in res.results], axis=0)


# revision 17
# speedup vs baseline: 1.3435x; 1.0320x over previous
"""ContraNorm Trainium2 kernel: out = 1.2*x - 0.2 * softmax(xn @ xn^T) @ x per batch.

Full input x [8, 2048, 512] f32; batch dim sharded across 8 NeuronCores
(data-parallel, no collectives). Each core runs an identical Bass/Tile program
on its [2048, 512] slice.

v4: exploits symmetry of sim = xn @ xn^T. Row-chunk orientation: chunk c
(rows 128c..128c+127 on partitions) computes sim columns b >= 128c only
(upper trapezoid, 144/256 blocks). The lower-left blocks are mirrors:
E2[:, j, cP:(c+1)P] = T(E2[:, c, jP:(j+1)P]) for j > c, produced by PE
matmul-transpose (lhsT = E-block, rhs = fp8 identity -> f32 PSUM) plus a
batched cast-copy back to fp8 SBUF. This halves both MM1 PE work and ACT
exp work (the two largest engine costs in the cost-model timeline).

Work is spread across engines to keep the setup and main-loop phases
balanced (cost-model gantt driven):
  setup: input DMAs alternate SP/ACT/DVE queues; ssq split ACT (Square +
  accum) / DVE (stt + accum); sqrt in two batches of 8 (halves the barrier);
  xn alternates DVE/ACT (Copy w/ per-partition scale); xnT copy-out round-
  robins DVE/ACT/Pool; xe cast (only needed by MM2) runs on Pool.
  main: exp on ACT; mirror copies alternate DVE/Pool; attn scale (po *
  -0.2/D) alternates ACT/DVE; combine on DVE.

Pools are created once and shared by all repeat bodies so repeated bodies
pipeline without per-body drains (steady-state measurement matches the
single-body program the grader runs).
"""

import sys

if "/opt/trn_rl_repo" not in sys.path:
    sys.path.insert(0, "/opt/trn_rl_repo")

from contextlib import ExitStack

import numpy as np

import concourse.bass as bass
import concourse.tile as tile
import concourse.mybir as mybir
from concourse import bacc
from concourse.masks import make_identity
from concourse.bass_utils import run_bass_kernel_spmd

F32 = mybir.dt.float32
BF16 = mybir.dt.bfloat16
FP8 = mybir.dt.float8e4
AF = mybir.ActivationFunctionType
ALU = mybir.AluOpType
DR = mybir.MatmulPerfMode.DoubleRow

B = 8
P = 128
N = 2048
D = 512
NT = N // P      # 16 row chunks
DS = D // P      # 4 d subtiles

VARIANT = ""  # debug bisect switches, comma-separated


def spans_for_chunk(c):
    """(b0, b1) spans covering [128c, 2048), each within one 512-f32 PSUM
    bank."""
    out = []
    b = 128 * c
    while b < N:
        e = min((b // 512 + 1) * 512, N)
        out.append((b, e))
        b = e
    return out


def make_pools(ctx: ExitStack, tc: tile.TileContext):
    pools = {}
    pools["singles"] = ctx.enter_context(tc.tile_pool(name="singles", bufs=1))
    pools["scratch"] = ctx.enter_context(tc.tile_pool(name="scratch", bufs=3))
    pools["stats"] = ctx.enter_context(tc.tile_pool(name="stats", bufs=8))
    pools["xnpool"] = ctx.enter_context(tc.tile_pool(name="xnpool", bufs=4))
    pools["tmppool"] = ctx.enter_context(tc.tile_pool(name="tmppool", bufs=3))
    pools["opool"] = ctx.enter_context(tc.tile_pool(name="opool", bufs=3))
    # PSUM budget (8 banks): psumS 4x1 (512-wide MM1+exp tiles; deep ring so
    # MM1 can run well ahead of the slower exp drain), psumM 2x1 (setup
    # transposes + mirror transposes, shared tag), psumO 1x2 (MM2 numerator +
    # denominator at col 512).
    pools["psumS"] = ctx.enter_context(tc.tile_pool(name="psumS", bufs=4, space="PSUM"))
    pools["psumM"] = ctx.enter_context(tc.tile_pool(name="psumM", bufs=2, space="PSUM"))
    pools["psumO"] = ctx.enter_context(tc.tile_pool(name="psumO", bufs=1, space="PSUM"))
    return pools


def contranorm_body(pools, tc: tile.TileContext, out_ap: bass.AP, x_ap: bass.AP,
                    first: bool):
    nc = tc.nc
    variants = set(VARIANT.split(","))

    singles = pools["singles"]
    scratch = pools["scratch"]
    stats = pools["stats"]
    xnpool = pools["xnpool"]
    tmppool = pools["tmppool"]
    opool = pools["opool"]
    psumS, psumM = pools["psumS"], pools["psumM"]
    psumO = pools["psumO"]

    # persistent tensors (same tiles every repeat; dependency-tracked)
    if first:
        xf = singles.tile([P, NT, D], F32, tag="xf")
        xe = singles.tile([P, NT, D + 16], FP8, tag="xe")
        xnT = singles.tile([P, DS, N], FP8, tag="xnT")
        E2 = singles.tile([P, NT, N], FP8, tag="E2")
        ssqA = singles.tile([P, NT], F32, tag="ssqA")
        nrmA = singles.tile([P, NT], F32, tag="nrmA")
        rnA = singles.tile([P, NT], F32, tag="rnA")
        identB = singles.tile([P, P], BF16, tag="identB")
        identE = singles.tile([P, P], FP8, tag="identE")
        pools.update(xf=xf, xe=xe, xnT=xnT, E2=E2, ssqA=ssqA, nrmA=nrmA,
                     rnA=rnA, identB=identB, identE=identE)
        make_identity(nc, identB)
        make_identity(nc, identE)
        nc.vector.memset(xe[:, :, D:D + 1], 1.0)
    xf, xe, xnT, E2 = pools["xf"], pools["xe"], pools["xnT"], pools["E2"]
    ssqA, nrmA, rnA = pools["ssqA"], pools["nrmA"], pools["rnA"]
    identB, identE = pools["identB"], pools["identE"]

    # ---------------- setup: norms, xn, transpose ----------------
    for i in range(NT):
        nc.sync.dma_start(xf[:, i, :], x_ap[i * P:(i + 1) * P, :])
        if i % 2 == 0:
            # ssq via ACT Square + accum_out
            sq = scratch.tile([P, D], F32, tag="sq")
            nc.scalar.activation(sq, xf[:, i, :], AF.Square,
                                 accum_out=ssqA[:, i:i + 1])
        else:
            # ssq via DVE: sq = (x * 1) * x, accum_out = sum(sq)
            sq = scratch.tile([P, D], F32, tag="sq")
            nc.vector.scalar_tensor_tensor(
                sq, xf[:, i, :], 1.0, xf[:, i, :], op0=ALU.mult, op1=ALU.mult,
                accum_out=ssqA[:, i:i + 1])
        # xe chunk (only needed once MM2 starts): Pool
        nc.gpsimd.tensor_copy(xe[:, i, 0:D], xf[:, i, :])
        if i == 7:
            nc.scalar.activation(nrmA[:, 0:8], ssqA[:, 0:8], AF.Sqrt)
            nc.vector.reciprocal(rnA[:, 0:8], nrmA[:, 0:8])
        elif i == 15:
            nc.scalar.activation(nrmA[:, 8:16], ssqA[:, 8:16], AF.Sqrt)
            nc.vector.reciprocal(rnA[:, 8:16], nrmA[:, 8:16])
    copy_engs = [nc.vector.tensor_copy, nc.scalar.copy]
    for i in range(NT):
        xn = xnpool.tile([P, D], BF16, tag="xn")
        if i % 2 == 0:
            nc.vector.tensor_scalar_mul(xn, xf[:, i, :], rnA[:, i:i + 1])
        else:
            nc.scalar.activation(xn, xf[:, i, :], AF.Copy, scale=rnA[:, i:i + 1])
        pt = psumM.tile([P, DS, P], BF16, tag="pm")
        for dc in range(DS):
            nc.tensor.transpose(pt[:, dc, :], xn[:, dc * P:(dc + 1) * P], identB)
        copy_engs[i % 2](xnT[:, :, i * P:(i + 1) * P], pt)

    # ---------------- main loop: one row-chunk c at a time ----------------
    mir_flip = 0
    for c in range(NT):
        # MM1 + exp over the trapezoid spans b in [128c, 2048)
        for (b0, b1) in spans_for_chunk(c):
            w = b1 - b0
            ps = psumS.tile([P, w], F32, tag="ps")
            for g in range(2):
                nc.tensor.matmul(
                    ps,
                    lhsT=xnT[:, 2 * g:2 * g + 2, c * P:(c + 1) * P],
                    rhs=xnT[:, 2 * g:2 * g + 2, b0:b1],
                    start=(g == 0), stop=(g == 1), perf_mode=DR)
            nc.scalar.activation(E2[:, c, b0:b1], ps, AF.Exp)
            # mirrors for the full blocks inside this span: j > c
            j0 = max(c + 1, (b0 + P - 1) // P)
            j1 = b1 // P
            for w0 in range(j0, j1, 4):
                w1 = min(w0 + 4, j1)
                nb = w1 - w0
                pm = psumM.tile([P, nb, P], F32, tag="pm")
                for t in range(nb):
                    j = w0 + t
                    nc.tensor.matmul(
                        pm[:, t, :],
                        lhsT=E2[:, c, j * P:(j + 1) * P],
                        rhs=identE, start=True, stop=True)
                if mir_flip % 2 == 0:
                    nc.vector.tensor_copy(E2[:, w0:w1, c * P:(c + 1) * P], pm)
                else:
                    nc.scalar.copy(E2[:, w0:w1, c * P:(c + 1) * P], pm)
                mir_flip += 1

        # MM2 for out row-tile h = c (all needed E2 slices now exist)
        h = c
        po = psumO.tile([P, 1024], F32, tag="po")  # [0:512]=num, [512]=denom
        for g in range(NT // 2):
            lhsT = E2[:, 2 * g:2 * g + 2, h * P:(h + 1) * P]
            nc.tensor.matmul(po[:, 0:D], lhsT, xe[:, 2 * g:2 * g + 2, 0:D],
                             start=(g == 0), stop=(g == NT // 2 - 1), perf_mode=DR)
            nc.tensor.matmul(po[:, D:D + 1], lhsT, xe[:, 2 * g:2 * g + 2, D:D + 1],
                             start=(g == 0), stop=(g == NT // 2 - 1), perf_mode=DR)
        # s = -0.2 / D
        sD = stats.tile([P, 1], F32, tag="sD")
        nc.vector.tensor_scalar_mul(sD, po[:, D:D + 1], -5.0)
        rD = stats.tile([P, 1], F32, tag="rD")
        nc.vector.reciprocal(rD, sD)
        # tmp = O * s (per-partition scale ptr, PSUM -> SBUF), ACT/DVE split
        tmp = tmppool.tile([P, D], F32, tag="tmp")
        if c % 2 == 0:
            nc.scalar.activation(tmp, po[:, 0:D], AF.Copy, scale=rD)
        else:
            nc.vector.tensor_scalar_mul(tmp, po[:, 0:D], rD)
        # out = x * 1.2 + tmp
        ob = opool.tile([P, D], F32, tag="ob")
        nc.vector.scalar_tensor_tensor(
            ob, xf[:, h, :], 1.2, tmp, op0=ALU.mult, op1=ALU.add)
        nc.scalar.dma_start(out_ap[h * P:(h + 1) * P, :], ob)


def build_nc(repeats: int = 1, loop: int = 0):
    """Build + compile the per-core Bass program. `repeats` re-emits the body
    (sharing pools/SBUF); `loop` wraps the body in a For_i hardware loop --
    both are for steady-state timing measurements."""
    nc = bacc.Bacc("TRN2", target_bir_lowering=False, debug=False, enable_asserts=False)
    x = nc.dram_tensor("x", [N, D], F32, kind="ExternalInput").ap()
    out = nc.dram_tensor("out", [N, D], F32, kind="ExternalOutput").ap()
    with tile.TileContext(nc) as tc:
        with ExitStack() as ctx:
            pools = make_pools(ctx, tc)
            if loop:
                with tc.For_i(0, loop, 1):
                    contranorm_body(pools, tc, out, x, first=True)
            else:
                for r in range(repeats):
                    contranorm_body(pools, tc, out, x, first=(r == 0))
    nc.compile()
    return nc


_nc_cache = {}


def kernel(x: np.ndarray) -> np.ndarray:
    assert x.shape == (B, N, D), x.shape
    x = np.ascontiguousarray(x, dtype=np.float32)
    if "nc" not in _nc_cache:
        _nc_cache["nc"] = build_nc()
    nc = _nc_cache["nc"]
    in_maps = [{"x": x[i]} for i in range(B)]
    res = run_bass_kernel_spmd(nc, in_maps, core_ids=list(range(B)))
    return np.stack([r["out"] for r in res.results], axis=0)


# revision 18
# speedup vs baseline: 1.6065x; 1.1958x over previous
"""ContraNorm Trainium2 kernel: out = 1.2*x - 0.2 * softmax(xn @ xn^T) @ x per batch.

Full input x [8, 2048, 512] f32; batch dim sharded across 8 NeuronCores
(data-parallel, no collectives). Each core runs an identical Bass/Tile program
on its [2048, 512] slice.

v4: exploits symmetry of sim = xn @ xn^T. Row-chunk orientation: chunk c
(rows 128c..128c+127 on partitions) computes sim columns b >= 128c only
(upper trapezoid, 144/256 blocks). The lower-left blocks are mirrors:
E2[:, j, cP:(c+1)P] = T(E2[:, c, jP:(j+1)P]) for j > c, produced by PE
matmul-transpose (lhsT = E-block, rhs = fp8 identity -> f32 PSUM) plus a
batched cast-copy back to fp8 SBUF. This halves both MM1 PE work and ACT
exp work (the two largest engine costs in the cost-model timeline).

Work is spread across engines to keep the setup and main-loop phases
balanced (cost-model gantt driven):
  setup: input DMAs alternate SP/ACT/DVE queues; ssq split ACT (Square +
  accum) / DVE (stt + accum); sqrt in two batches of 8 (halves the barrier);
  xn alternates DVE/ACT (Copy w/ per-partition scale); xnT copy-out round-
  robins DVE/ACT/Pool; xe cast (only needed by MM2) runs on Pool.
  main: exp on ACT; mirror copies alternate DVE/Pool; attn scale (po *
  -0.2/D) alternates ACT/DVE; combine on DVE.

Pools are created once and shared by all repeat bodies so repeated bodies
pipeline without per-body drains (steady-state measurement matches the
single-body program the grader runs).
"""

import sys

if "/opt/trn_rl_repo" not in sys.path:
    sys.path.insert(0, "/opt/trn_rl_repo")

from contextlib import ExitStack

import numpy as np

import concourse.bass as bass
import concourse.tile as tile
import concourse.mybir as mybir
from concourse import bacc
from concourse.masks import make_identity
from concourse.bass_utils import run_bass_kernel_spmd

F32 = mybir.dt.float32
BF16 = mybir.dt.bfloat16
FP8 = mybir.dt.float8e4
AF = mybir.ActivationFunctionType
ALU = mybir.AluOpType
DR = mybir.MatmulPerfMode.DoubleRow

B = 8
P = 128
N = 2048
D = 512
NT = N // P      # 16 row chunks
DS = D // P      # 4 d subtiles

VARIANT = ""  # debug bisect switches, comma-separated


def spans_for_chunk(c):
    """(b0, b1) spans covering [128c, 2048), each within one 512-f32 PSUM
    bank."""
    out = []
    b = 128 * c
    while b < N:
        e = min((b // 512 + 1) * 512, N)
        out.append((b, e))
        b = e
    return out


def make_pools(ctx: ExitStack, tc: tile.TileContext):
    pools = {}
    pools["singles"] = ctx.enter_context(tc.tile_pool(name="singles", bufs=1))
    pools["scratch"] = ctx.enter_context(tc.tile_pool(name="scratch", bufs=3))
    pools["stats"] = ctx.enter_context(tc.tile_pool(name="stats", bufs=8))
    pools["xnpool"] = ctx.enter_context(tc.tile_pool(name="xnpool", bufs=4))
    pools["tmppool"] = ctx.enter_context(tc.tile_pool(name="tmppool", bufs=3))
    pools["opool"] = ctx.enter_context(tc.tile_pool(name="opool", bufs=3))
    # PSUM budget (8 banks): psumS 4x1 (512-wide MM1+exp tiles; deep ring so
    # MM1 can run well ahead of the slower exp drain), psumM 2x1 (setup
    # transposes + mirror transposes, shared tag), psumO 1x2 (MM2 numerator +
    # denominator at col 512).
    pools["psumS"] = ctx.enter_context(tc.tile_pool(name="psumS", bufs=2, space="PSUM"))
    pools["psumM"] = ctx.enter_context(tc.tile_pool(name="psumM", bufs=2, space="PSUM"))
    pools["psumO"] = ctx.enter_context(tc.tile_pool(name="psumO", bufs=1, space="PSUM"))
    return pools


def contranorm_body(pools, tc: tile.TileContext, out_ap: bass.AP, x_ap: bass.AP,
                    first: bool):
    nc = tc.nc
    variants = set(VARIANT.split(","))

    singles = pools["singles"]
    scratch = pools["scratch"]
    stats = pools["stats"]
    xnpool = pools["xnpool"]
    tmppool = pools["tmppool"]
    opool = pools["opool"]
    psumS, psumM = pools["psumS"], pools["psumM"]
    psumO = pools["psumO"]

    # persistent tensors (same tiles every repeat; dependency-tracked)
    if first:
        xf = singles.tile([P, NT, D], F32, tag="xf")
        xe = singles.tile([P, NT, D + 16], FP8, tag="xe")
        xnT = singles.tile([P, DS, N], FP8, tag="xnT")
        E2 = singles.tile([P, NT, N], FP8, tag="E2")
        ssqA = singles.tile([P, NT], F32, tag="ssqA")
        nrmA = singles.tile([P, NT], F32, tag="nrmA")
        rnA = singles.tile([P, NT], F32, tag="rnA")
        identB = singles.tile([P, P], BF16, tag="identB")
        identE = singles.tile([P, P], FP8, tag="identE")
        pools.update(xf=xf, xe=xe, xnT=xnT, E2=E2, ssqA=ssqA, nrmA=nrmA,
                     rnA=rnA, identB=identB, identE=identE)
        make_identity(nc, identB)
        make_identity(nc, identE)
        nc.vector.memset(xe[:, :, D:D + 1], 1.0)
    xf, xe, xnT, E2 = pools["xf"], pools["xe"], pools["xnT"], pools["E2"]
    ssqA, nrmA, rnA = pools["ssqA"], pools["nrmA"], pools["rnA"]
    identB, identE = pools["identB"], pools["identE"]

    # ---------------- setup: norms, xn, transpose ----------------
    for i in range(NT):
        nc.sync.dma_start(xf[:, i, :], x_ap[i * P:(i + 1) * P, :])
        if i % 2 == 0:
            # ssq via ACT Square + accum_out
            sq = scratch.tile([P, D], F32, tag="sq")
            nc.scalar.activation(sq, xf[:, i, :], AF.Square,
                                 accum_out=ssqA[:, i:i + 1])
        else:
            # ssq via DVE: sq = (x * 1) * x, accum_out = sum(sq)
            sq = scratch.tile([P, D], F32, tag="sq")
            nc.vector.scalar_tensor_tensor(
                sq, xf[:, i, :], 1.0, xf[:, i, :], op0=ALU.mult, op1=ALU.mult,
                accum_out=ssqA[:, i:i + 1])
        # xe chunk (only needed once MM2 starts): Pool
        nc.gpsimd.tensor_copy(xe[:, i, 0:D], xf[:, i, :])
        if i == 7:
            nc.scalar.activation(nrmA[:, 0:8], ssqA[:, 0:8], AF.Sqrt)
            nc.vector.reciprocal(rnA[:, 0:8], nrmA[:, 0:8])
        elif i == 15:
            nc.scalar.activation(nrmA[:, 8:16], ssqA[:, 8:16], AF.Sqrt)
            nc.vector.reciprocal(rnA[:, 8:16], nrmA[:, 8:16])
    copy_engs = [nc.vector.tensor_copy, nc.scalar.copy]
    for i in range(NT):
        xn = xnpool.tile([P, D], BF16, tag="xn")
        if i % 2 == 0:
            nc.vector.tensor_scalar_mul(xn, xf[:, i, :], rnA[:, i:i + 1])
        else:
            nc.scalar.activation(xn, xf[:, i, :], AF.Copy, scale=rnA[:, i:i + 1])
        pt = psumM.tile([P, DS, P], BF16, tag="pm")
        for dc in range(DS):
            nc.tensor.transpose(pt[:, dc, :], xn[:, dc * P:(dc + 1) * P], identB)
        copy_engs[i % 2](xnT[:, :, i * P:(i + 1) * P], pt)

    # ---------------- main loop: one row-chunk c at a time ----------------
    mir_flip = 0
    for c in range(NT):
        # MM1 + exp over the trapezoid b in [128c, 2048), two 512-banks per
        # PSUM tile. Matmul halves start at 512-aligned a0 <= 128c (the few
        # columns below 128c are computed but never read); exp covers the
        # valid contiguous tail of the tile in one instruction.
        a0 = (128 * c // 512) * 512
        for t0 in range(a0, N, 1024):
            t1 = min(t0 + 1024, N)
            ps = psumS.tile([P, t1 - t0], F32, tag="ps")
            for (s0, s1) in ((t0, min(t0 + 512, t1)), (t0 + 512, t1)):
                if s0 >= s1:
                    continue
                for g in range(2):
                    nc.tensor.matmul(
                        ps[:, s0 - t0:s1 - t0],
                        lhsT=xnT[:, 2 * g:2 * g + 2, c * P:(c + 1) * P],
                        rhs=xnT[:, 2 * g:2 * g + 2, s0:s1],
                        start=(g == 0), stop=(g == 1), perf_mode=DR)
            b0 = max(t0, 128 * c)  # valid region start
            nc.scalar.activation(E2[:, c, b0:t1], ps[:, b0 - t0:], AF.Exp)
            # mirrors for the full blocks inside this tile: j > c
            j0 = max(c + 1, (b0 + P - 1) // P)
            j1 = t1 // P
            for w0 in range(j0, j1, 4):
                w1 = min(w0 + 4, j1)
                nb = w1 - w0
                pm = psumM.tile([P, nb, P], F32, tag="pm")
                for t in range(nb):
                    j = w0 + t
                    nc.tensor.matmul(
                        pm[:, t, :],
                        lhsT=E2[:, c, j * P:(j + 1) * P],
                        rhs=identE, start=True, stop=True)
                if mir_flip % 2 == 0:
                    nc.vector.tensor_copy(E2[:, w0:w1, c * P:(c + 1) * P], pm)
                else:
                    nc.scalar.copy(E2[:, w0:w1, c * P:(c + 1) * P], pm)
                mir_flip += 1

        # MM2 for out row-tile h = c (all needed E2 slices now exist)
        h = c
        po = psumO.tile([P, 1024], F32, tag="po")  # [0:512]=num, [512]=denom
        for g in range(NT // 2):
            lhsT = E2[:, 2 * g:2 * g + 2, h * P:(h + 1) * P]
            nc.tensor.matmul(po[:, 0:D], lhsT, xe[:, 2 * g:2 * g + 2, 0:D],
                             start=(g == 0), stop=(g == NT // 2 - 1), perf_mode=DR)
            nc.tensor.matmul(po[:, D:D + 1], lhsT, xe[:, 2 * g:2 * g + 2, D:D + 1],
                             start=(g == 0), stop=(g == NT // 2 - 1), perf_mode=DR)
        # s = -0.2 / D
        sD = stats.tile([P, 1], F32, tag="sD")
        nc.vector.tensor_scalar_mul(sD, po[:, D:D + 1], -5.0)
        rD = stats.tile([P, 1], F32, tag="rD")
        nc.vector.reciprocal(rD, sD)
        # tmp = O * s (per-partition scale ptr, PSUM -> SBUF), ACT/DVE split
        tmp = tmppool.tile([P, D], F32, tag="tmp")
        nc.vector.tensor_scalar_mul(tmp, po[:, 0:D], rD)
        # out = x * 1.2 + tmp
        ob = opool.tile([P, D], F32, tag="ob")
        nc.vector.scalar_tensor_tensor(
            ob, xf[:, h, :], 1.2, tmp, op0=ALU.mult, op1=ALU.add)
        (nc.sync if h % 2 == 0 else nc.scalar).dma_start(
            out_ap[h * P:(h + 1) * P, :], ob)


def build_nc(repeats: int = 1, loop: int = 0):
    """Build + compile the per-core Bass program. `repeats` re-emits the body
    (sharing pools/SBUF); `loop` wraps the body in a For_i hardware loop --
    both are for steady-state timing measurements."""
    nc = bacc.Bacc("TRN2", target_bir_lowering=False, debug=False, enable_asserts=False)
    x = nc.dram_tensor("x", [N, D], F32, kind="ExternalInput").ap()
    out = nc.dram_tensor("out", [N, D], F32, kind="ExternalOutput").ap()
    with tile.TileContext(nc) as tc:
        with ExitStack() as ctx:
            pools = make_pools(ctx, tc)
            if loop:
                with tc.For_i(0, loop, 1):
                    contranorm_body(pools, tc, out, x, first=True)
            else:
                for r in range(repeats):
                    contranorm_body(pools, tc, out, x, first=(r == 0))
    nc.compile()
    return nc


_nc_cache = {}


def kernel(x: np.ndarray) -> np.ndarray:
    assert x.shape == (B, N, D), x.shape
    x = np.ascontiguousarray(x, dtype=np.float32)
    if "nc" not in _nc_cache:
        _nc_cache["nc"] = build_nc()
    nc = _nc_cache["nc"]
    in_maps = [{"x": x[i]} for i in range(B)]
    res = run_bass_kernel_spmd(nc, in_maps, core_ids=list(range(B)))
    return np.stack([r["out"] for r in res.results], axis=0)
